# revision 3
# baseline (speedup 1.0000x reference)
"""GQA causal attention (S=2048, H=32, KVH=8, D=128) on 8 TRN2 NeuronCores.

Sharding: tensor-parallel over heads. Core i computes query heads
[4i, 4i+4) against KV head i (GQA group size 32/8 = 4). No collectives:
the host slices the inputs per core and concatenates the outputs.

Per-core algorithm (seq=2048, d=128, 4 q-heads, 1 kv-head, causal):
  - K^T and per-head Q^T are produced WITHOUT compute engines:
    SWDGE DMA casts fp32->bf16 (DRAM->DRAM), then an XBAR-transpose DMA
    lands [d=128, seq] bf16 directly in SBUF.
  - Per head, the exact-causal score tiles S^T[kt] = K_tile^T @ Q^T
    (only q >= kt*128) are written PACKED into alternating PSUM buffers
    A[128,2048] / B[128,1536]; ONE wide ACTIVATE(Exp, scale) per buffer
    writes the packed P^T row [128, 17408] bf16 in SBUF (scores are O(1)
    so no max subtraction). 40 activations total instead of 96.
  - The diagonal 128-col block of each key-tile region is masked by a
    0/1 upper-triangular multiply on the DVE.
  - PV: for each query tile qt, acc[qt] = sum_k2 (P^T slice).T @ [V | 1]
    accumulated in PSUM; column 128 is the softmax denominator.
    DVE reciprocal + tensor_scalar_mul normalizes; one DMA per 256 rows
    stores the result. PV lags the QK/exp pipeline by a few tiles and
    flows across head boundaries so no engine sees a bubble.
"""

import numpy as np

SEQ = 2048
D = 128
QH = 4  # query heads per core
N_CORES = 8
SCALE = 0.08838834764831845  # 1/sqrt(128)
NT = SEQ // 128  # 16 tiles of 128 along seq

_NC = None

# packed score-column layout (identical per head)
ROFF = [0]
for _kt in range(1, NT + 1):
    ROFF.append(ROFF[-1] + (SEQ - 128 * (_kt - 1)))
PCOLS = ROFF[NT]  # 17408

# psum buffers: alternate A(2048) / B(1536), truncate last
BUFS = []
_c = 0
_i = 0
while _c < PCOLS:
    sz = 2048 if _i % 2 == 0 else 1536
    sz = min(sz, PCOLS - _c)
    BUFS.append((_c, sz, _i % 2))  # (start col, size, which pool)
    _c += sz
    _i += 1


def _emit(ctx, tc, q, k, v, out):
    import concourse.mybir as mybir
    from concourse import masks

    nc = tc.nc
    f32 = mybir.dt.float32
    bf16 = mybir.dt.bfloat16
    Exp = mybir.ActivationFunctionType.Exp

    singles = ctx.enter_context(tc.tile_pool(name="singles", bufs=1))
    qpool = ctx.enter_context(tc.tile_pool(name="qpool", bufs=2))
    ppool = ctx.enter_context(tc.tile_pool(name="ppool", bufs=2))
    opool = ctx.enter_context(tc.tile_pool(name="opool", bufs=3))
    # PSUM budget (8 banks = 16KB/partition):
    #   A 2048 f32 = 4 banks, B 1536 f32 = 3 banks, PV acc [128,3,129] = 1 bank
    psum_a = ctx.enter_context(tc.tile_pool(name="psum_a", bufs=1, space="PSUM"))
    psum_b = ctx.enter_context(tc.tile_pool(name="psum_b", bufs=1, space="PSUM"))
    psum_o = ctx.enter_context(tc.tile_pool(name="psum_o", bufs=1, space="PSUM"))

    sA = psum_a.tile([128, 2048], f32, tag="A")
    sB = psum_b.tile([128, 1536], f32, tag="B")
    ops_tri = psum_o.tile([128, 3, D + 1], f32, tag="o")

    # ---- PE warmup: dense dummy matmuls while the DMA prep runs, so the
    # HAM clock-gate reaches 2.4 GHz by the time real PE work arrives.
    warm_src = singles.tile([128, 512], bf16, tag="warm_src")
    nc.vector.memset(warm_src[:], 0.0)
    for _ in range(16):
        nc.tensor.matmul(
            sB[:, 0:512], lhsT=warm_src[:, 0:128], rhs=warm_src[:], start=True, stop=True
        )

    keep = singles.tile([128, 128], bf16)
    masks.make_upper_triangular(nc, keep[:], val=1.0, diag=True)

    # ---- K/Q prep: SWDGE cast DMA (fp32 DRAM -> bf16 DRAM scratch), then
    # XBAR transpose DMA (bf16 DRAM -> SBUF [d, seq]). No PE/DVE involved.
    kT = singles.tile([128, SEQ], bf16, tag="kT")
    k_sc = nc.dram_tensor("k_sc", [SEQ, D], bf16)

    def kprep(c):  # rows [512c, 512c+512)
        rs = slice(512 * c, 512 * (c + 1))
        nc.gpsimd.dma_start(out=k_sc[rs, :], in_=k[rs, :])
        nc.sync.dma_start(out=kT[:, rs], in_=k_sc[rs, :], transpose=True)

    q_sc = [nc.dram_tensor(f"q_sc{h}", [SEQ, D], bf16) for h in range(QH)]

    def qprep_cast(h, c):
        rs = slice(512 * c, 512 * (c + 1))
        nc.gpsimd.dma_start(out=q_sc[h][rs, :], in_=q[rs, h * D:(h + 1) * D])

    def qprep_tp(h, qT, c):
        rs = slice(512 * c, 512 * (c + 1))
        nc.sync.dma_start(out=qT[:, rs], in_=q_sc[h][rs, :], transpose=True)

    # interleave K and head-0 Q so the first QK matmul is reachable fast
    qT = qpool.tile([128, SEQ], bf16, tag="qT")
    kprep(0)
    qprep_cast(0, 0)
    qprep_tp(0, qT, 0)
    for c in range(1, 4):
        kprep(c)
        qprep_cast(0, c)
        qprep_tp(0, qT, c)

    # ---- V: natural [128, t, d] bf16 + ones column for the denominator,
    # cast in-flight by SWDGE.
    vp = singles.tile([128, NT, D + 1], bf16)
    nc.gpsimd.dma_start(
        out=vp[:, :, 0:D], in_=v.rearrange("(t p) d -> p t d", p=128)
    )
    nc.vector.memset(vp[:, :, D:D + 1], 1.0)

    def emit_pv(h, qt, pT, osb):
        """O[qt] = sum_k2 (P^T slice).T @ [V | 1], then normalize + store."""
        ops = ops_tri[:, qt % 3, :]
        for k2 in range(qt + 1):
            c0 = ROFF[k2] + (qt - k2) * 128
            nc.tensor.matmul(
                ops,
                lhsT=pT[:, c0:c0 + 128],
                rhs=vp[:, k2, :],
                start=(k2 == 0),
                stop=(k2 == qt),
            )
        rec = opool.tile([128, 1], f32, tag="rec")
        nc.vector.reciprocal(rec[:], ops[:, D:D + 1])
        nc.vector.tensor_scalar_mul(osb[:, qt % 2, :], ops[:, 0:D], rec[:])
        if qt % 2 == 1:
            qb = qt // 2
            nc.sync.dma_start(
                out=out[qb * 256:(qb + 1) * 256, h * D:(h + 1) * D].rearrange(
                    "(j p) d -> p j d", p=128
                ),
                in_=osb[:],
            )

    # Pending-PV queue, flowing across head boundaries.
    pvq = []
    pv_state = {}

    def pop_pv():
        h2, qt2, pT2 = pvq.pop(0)
        st = pv_state.setdefault(h2, {})
        if qt2 % 2 == 0:
            st["osb"] = opool.tile([128, 2, D], f32, tag="osb", name="osb")
        emit_pv(h2, qt2, pT2, st["osb"])

    LAG = 3

    def region_of(c):
        # region kt containing packed column c
        kt = 0
        while ROFF[kt + 1] <= c:
            kt += 1
        return kt

    for h in range(QH):
        pT = ppool.tile([128, PCOLS], bf16, tag="pT")
        qT_next = None
        next_qt = 0  # next query tile to mark PV-ready
        for bi, (b0, bsz, which) in enumerate(BUFS):
            sbuf_tile = sA if which == 0 else sB
            # exact-causal QK chunks packed into this psum buffer
            c = b0
            while c < b0 + bsz:
                kt = region_of(c)
                qoff = kt * 128 + (c - ROFF[kt])  # query index of col c
                step = min(
                    512 - (c - b0) % 512,  # psum bank grid
                    ROFF[kt + 1] - c,      # region end
                    b0 + bsz - c,          # buffer end
                )
                nc.tensor.matmul(
                    sbuf_tile[:, c - b0:c - b0 + step],
                    lhsT=kT[:, kt * 128:(kt + 1) * 128],
                    rhs=qT[:, qoff:qoff + step],
                    start=True,
                    stop=True,
                )
                c += step
            # one wide exp for the whole buffer
            nc.scalar.activation(
                pT[:, b0:b0 + bsz], sbuf_tile[:, 0:bsz], Exp, scale=SCALE
            )
            # mask any diagonal block that this buffer completed
            kt = region_of(b0)
            while kt < NT and ROFF[kt] + 128 <= b0 + bsz:
                if ROFF[kt] + 128 > b0:
                    nc.vector.tensor_mul(
                        pT[:, ROFF[kt]:ROFF[kt] + 128],
                        pT[:, ROFF[kt]:ROFF[kt] + 128],
                        keep[:],
                    )
                kt += 1
            # queue query tiles whose last dependency (diag block) is done
            while next_qt < NT and ROFF[next_qt] + 128 <= b0 + bsz:
                pvq.append((h, next_qt, pT))
                next_qt += 1
            while len(pvq) > LAG:
                pop_pv()
            # prefetch next head's Q (cast early, transpose later)
            if h + 1 < QH:
                if bi == 2:
                    for cc in range(4):
                        qprep_cast(h + 1, cc)
                elif bi == 5:
                    qT_next = qpool.tile([128, SEQ], bf16, tag="qT")
                    qprep_tp(h + 1, qT_next, 0)
                    qprep_tp(h + 1, qT_next, 1)
                elif bi == 7:
                    qprep_tp(h + 1, qT_next, 2)
                    qprep_tp(h + 1, qT_next, 3)
        if qT_next is not None:
            qT = qT_next
    while pvq:
        pop_pv()


def _build():
    import concourse.mybir as mybir
    import concourse.tile as tile
    from concourse import bacc
    from contextlib import ExitStack

    nc = bacc.Bacc()
    q = nc.declare_dram_parameter("q", [SEQ, QH * D], mybir.dt.float32, isOutput=False)
    k = nc.declare_dram_parameter("k", [SEQ, D], mybir.dt.float32, isOutput=False)
    v = nc.declare_dram_parameter("v", [SEQ, D], mybir.dt.float32, isOutput=False)
    out = nc.declare_dram_parameter("out", [SEQ, QH * D], mybir.dt.float32, isOutput=True)

    with tile.TileContext(nc) as tc:
        with ExitStack() as ctx:
            _emit(ctx, tc, q[:], k[:], v[:], out[:])
    nc.compile()
    return nc


def _get_nc():
    global _NC
    if _NC is None:
        _NC = _build()
    return _NC


def _ensure_ntff_hook():
    """The agent image's antenv lacks axon_hooks; shim it so trace=True works."""
    import sys
    import types

    if "antenv.axon_hooks" in sys.modules:
        return
    try:
        import antenv
        from trn_agent_boot.trn_boot import _ntff_profile_via_ctypes
    except ImportError:
        return
    mod = types.ModuleType("antenv.axon_hooks")
    hook = [None]
    mod.set_axon_ntff_profile_hook = lambda h: hook.__setitem__(0, h)
    mod.get_axon_ntff_profile_hook = lambda: hook[0]
    sys.modules["antenv.axon_hooks"] = mod
    antenv.axon_hooks = mod
    mod.set_axon_ntff_profile_hook(_ntff_profile_via_ctypes("/opt/axon/libaxon_pjrt.so"))


def _run(q, k, v, trace=False):
    from concourse.bass_utils import run_bass_kernel_spmd

    if trace:
        _ensure_ntff_hook()
    nc = _get_nc()
    in_maps = []
    for i in range(N_CORES):
        in_maps.append(
            {
                "q": np.ascontiguousarray(q[:, i * QH * D:(i + 1) * QH * D]).astype(np.float32, copy=False),
                "k": np.ascontiguousarray(k[:, i * D:(i + 1) * D]).astype(np.float32, copy=False),
                "v": np.ascontiguousarray(v[:, i * D:(i + 1) * D]).astype(np.float32, copy=False),
            }
        )
    res = run_bass_kernel_spmd(nc, in_maps, core_ids=list(range(N_CORES)), trace=trace)
    full = np.concatenate([res.results[i]["out"] for i in range(N_CORES)], axis=1)
    return full.astype(np.float32, copy=False), res


def kernel(q, k, v):
    out, _ = _run(q, k, v, trace=False)
    return out


# revision 6
# speedup vs baseline: 1.1615x; 1.1615x over previous
"""GQA causal attention (S=2048, H=32, KVH=8, D=128) on 8 TRN2 NeuronCores.

Sharding: tensor-parallel over heads. Core i computes query heads
[4i, 4i+4) against KV head i (GQA group size 32/8 = 4). No collectives:
the host slices the inputs per core and concatenates the outputs.

Per-core algorithm (seq=2048, d=128, 4 q-heads, 1 kv-head, causal):
  - K^T and head-0 Q^T take the low-latency prep path: fp32 DMA load,
    DVE cast to bf16, PE identity-matmul transpose (chunked, interleaved
    with warmup matmuls that hold the HAM clock at 2.4 GHz).
  - Q^T for heads 1-3 is produced entirely by DMA engines in the
    background: a SWDGE DMA casts fp32->bf16 (DRAM->DRAM), then an
    XBAR-transpose DMA lands [d=128, seq] bf16 in SBUF. These are issued
    at t=0 and consumed 25+ us later.
  - Per head, exact-causal score tiles S^T[kt] = K_tile^T @ Q^T (only
    q >= kt*128) are written PACKED into alternating PSUM buffers
    A[128,2048] / B[128,1024]; ONE wide ACTIVATE(Exp, scale) per buffer
    writes into the packed P^T buffer [128, 17408] bf16 (scores are O(1)
    so no max subtraction). 44 activations instead of 96.
  - The diagonal 128-col block of each key-tile region is masked by a
    0/1 upper-triangular multiply on the (otherwise idle) GpSimd engine.
  - PV: for each query tile qt, acc[qt] = sum_k2 (P^T slice).T @ [V | 1]
    accumulated in PSUM; column 128 is the softmax denominator.
    DVE reciprocal + tensor_scalar_mul normalizes; one DMA per 256 rows
    stores the result. PV lags the QK/exp pipeline by a few query tiles
    and flows across head boundaries so no engine sees a bubble.
"""

import numpy as np

SEQ = 2048
D = 128
QH = 4  # query heads per core
N_CORES = 8
SCALE = 0.08838834764831845  # 1/sqrt(128)
NT = SEQ // 128  # 16 tiles of 128 along seq

_NC = None

# packed score-column layout (identical per head)
ROFF = [0]
for _kt in range(1, NT + 1):
    ROFF.append(ROFF[-1] + (SEQ - 128 * (_kt - 1)))
PCOLS = ROFF[NT]  # 17408

# psum buffers: alternate A(2048) / B(1024), truncate last
BUFS = []
_c = 0
_i = 0
while _c < PCOLS:
    sz = 2048 if _i % 2 == 0 else 1024
    sz = min(sz, PCOLS - _c)
    BUFS.append((_c, sz, _i % 2))  # (start col, size, which pool)
    _c += sz
    _i += 1


def _emit(ctx, tc, q, k, v, out):
    import concourse.mybir as mybir
    from concourse import masks

    nc = tc.nc
    f32 = mybir.dt.float32
    bf16 = mybir.dt.bfloat16
    Exp = mybir.ActivationFunctionType.Exp

    singles = ctx.enter_context(tc.tile_pool(name="singles", bufs=1))
    ppool = ctx.enter_context(tc.tile_pool(name="ppool", bufs=2))
    opool = ctx.enter_context(tc.tile_pool(name="opool", bufs=3))
    # PSUM budget (8 banks = 16KB/partition):
    #   A 2048 f32 = 4 banks, B 1024 f32 = 2 banks,
    #   PV acc [128,2,129] f32 = 1 bank, transpose staging = 1 bank
    psum_a = ctx.enter_context(tc.tile_pool(name="psum_a", bufs=1, space="PSUM"))
    psum_b = ctx.enter_context(tc.tile_pool(name="psum_b", bufs=1, space="PSUM"))
    psum_o = ctx.enter_context(tc.tile_pool(name="psum_o", bufs=1, space="PSUM"))
    psum_t = ctx.enter_context(tc.tile_pool(name="psum_t", bufs=1, space="PSUM"))

    sA = psum_a.tile([128, 2048], f32, tag="A")
    sB = psum_b.tile([128, 1024], f32, tag="B")
    ops_tri = psum_o.tile([128, 2, D + 1], f32, tag="o")
    # two transpose staging slots inside one PSUM bank (slices rotate)
    tps = psum_t.tile([128, 2, 128], bf16, tag="tp")

    # ---- PE warmup: dummy matmuls so the HAM clock-gate reaches 2.4 GHz
    # by the time real PE work arrives (identity transposes don't count).
    warm_src = singles.tile([128, 512], bf16, tag="warm_src")
    nc.vector.memset(warm_src[:], 0.0)

    def warm(n):
        for _ in range(n):
            nc.tensor.matmul(
                sB[:, 0:512], lhsT=warm_src[:, 0:128], rhs=warm_src[:],
                start=True, stop=True,
            )

    warm(10)

    ident = singles.tile([128, 128], bf16)
    masks.make_identity(nc, ident[:])
    keep = singles.tile([128, 128], bf16)
    masks.make_upper_triangular(nc, keep[:], val=1.0, diag=True)

    qT = [None] * QH
    # ---- background prep for heads 1-3: SWDGE cast DMA then XBAR
    # transpose DMA; no compute engines involved.
    q_sc = [None] * QH
    for h in range(1, QH):
        q_sc[h] = nc.dram_tensor(f"q_sc{h}", [SEQ, D], bf16)
        nc.gpsimd.dma_start(out=q_sc[h][:, :], in_=q[:, h * D:(h + 1) * D])

    # ---- fast-path prep: K and head-0 Q via load + DVE cast + PE transpose
    kT = singles.tile([128, SEQ], bf16, tag="kT")
    knat = singles.tile([128, NT, 128], f32, tag="knat")
    knat_bf = singles.tile([128, NT, 128], bf16, tag="knat_bf")
    kr = k.rearrange("(t p) d -> p t d", p=128)
    qT[0] = singles.tile([128, SEQ], bf16, tag="qT0", name="qT0")
    q0nat = singles.tile([128, NT, 128], f32, tag="q0nat")
    q0nat_bf = singles.tile([128, NT, 128], bf16, tag="q0nat_bf")
    q0r = q[:, 0:D].rearrange("(t p) d -> p t d", p=128)

    def prep_chunk(c, nat, nat_bf, dst, src):
        cs = slice(c * 4, (c + 1) * 4)
        nc.sync.dma_start(out=nat[:, cs, :], in_=src[:, cs, :])
        nc.vector.tensor_copy(nat_bf[:, cs, :], nat[:, cs, :])
        for t in range(c * 4, (c + 1) * 4):
            pst = tps[:, t % 2, :]
            nc.tensor.transpose(pst, nat_bf[:, t, :], ident[:])
            nc.vector.tensor_copy(dst[:, t * 128:(t + 1) * 128], pst)

    # ---- V: natural [128, t, d] bf16 + ones column for the denominator
    vp = singles.tile([128, NT, D + 1], bf16)
    vnat = singles.tile([128, NT, 128], f32, tag="vnat")
    vr = v.rearrange("(t p) d -> p t d", p=128)

    for c in range(4):
        prep_chunk(c, knat, knat_bf, kT, kr)
        warm(1)
        prep_chunk(c, q0nat, q0nat_bf, qT[0], q0r)
        warm(1)
        if c < 2:
            cs = slice(c * 8, (c + 1) * 8)
            nc.sync.dma_start(out=vnat[:, cs, :], in_=vr[:, cs, :])
            nc.vector.tensor_copy(vp[:, cs, 0:D], vnat[:, cs, :])
    nc.vector.memset(vp[:, :, D:D + 1], 1.0)

    # heads 1-3: XBAR transpose straight into SBUF (waits on the cast DMA)
    for h in range(1, QH):
        qT[h] = singles.tile([128, SEQ], bf16, tag=f"qT{h}", name=f"qT{h}")
        nc.sync.dma_start(out=qT[h][:, :], in_=q_sc[h][:, :], transpose=True)

    def emit_pv(h, qt, pT, osb):
        """O[qt] = sum_k2 (P^T slice).T @ [V | 1], then normalize + store."""
        ops = ops_tri[:, qt % 2, :]
        for k2 in range(qt + 1):
            c0 = ROFF[k2] + (qt - k2) * 128
            nc.tensor.matmul(
                ops,
                lhsT=pT[:, c0:c0 + 128],
                rhs=vp[:, k2, :],
                start=(k2 == 0),
                stop=(k2 == qt),
            )
        rec = opool.tile([128, 1], f32, tag="rec")
        nc.vector.reciprocal(rec[:], ops[:, D:D + 1])
        nc.vector.tensor_scalar_mul(osb[:, qt % 2, :], ops[:, 0:D], rec[:])
        if qt % 2 == 1:
            qb = qt // 2
            nc.sync.dma_start(
                out=out[qb * 256:(qb + 1) * 256, h * D:(h + 1) * D].rearrange(
                    "(j p) d -> p j d", p=128
                ),
                in_=osb[:],
            )

    # Pending-PV queue, flowing across head boundaries.
    pvq = []
    pv_state = {}

    def pop_pv():
        h2, qt2, pT2 = pvq.pop(0)
        st = pv_state.setdefault(h2, {})
        if qt2 % 2 == 0:
            st["osb"] = opool.tile([128, 2, D], f32, tag="osb", name="osb")
        emit_pv(h2, qt2, pT2, st["osb"])

    LAG = 4

    def region_of(c):
        kt = 0
        while ROFF[kt + 1] <= c:
            kt += 1
        return kt

    for h in range(QH):
        pT = ppool.tile([128, PCOLS], bf16, tag="pT")
        next_qt = 0  # next query tile to mark PV-ready
        for b0, bsz, which in BUFS:
            # drain PV backlog first: gives the scalar engine time to free
            # the psum buffer this iteration is about to overwrite
            while len(pvq) > LAG:
                pop_pv()
            sbuf_tile = sA if which == 0 else sB
            # exact-causal QK chunks packed into this psum buffer
            c = b0
            while c < b0 + bsz:
                kt = region_of(c)
                qoff = kt * 128 + (c - ROFF[kt])  # query index of col c
                step = min(
                    512 - (c - b0) % 512,  # psum bank grid
                    ROFF[kt + 1] - c,      # region end
                    b0 + bsz - c,          # buffer end
                )
                nc.tensor.matmul(
                    sbuf_tile[:, c - b0:c - b0 + step],
                    lhsT=kT[:, kt * 128:(kt + 1) * 128],
                    rhs=qT[h][:, qoff:qoff + step],
                    start=True,
                    stop=True,
                )
                c += step
            # one wide exp for the whole buffer
            nc.scalar.activation(
                pT[:, b0:b0 + bsz], sbuf_tile[:, 0:bsz], Exp, scale=SCALE
            )
            # mask any diagonal block this buffer completed (on GpSimd)
            kt = region_of(b0)
            while kt < NT and ROFF[kt] + 128 <= b0 + bsz:
                if ROFF[kt] + 128 > b0:
                    nc.gpsimd.tensor_mul(
                        pT[:, ROFF[kt]:ROFF[kt] + 128],
                        pT[:, ROFF[kt]:ROFF[kt] + 128],
                        keep[:],
                    )
                kt += 1
            # queue query tiles whose last dependency (diag block) is done
            while next_qt < NT and ROFF[next_qt] + 128 <= b0 + bsz:
                pvq.append((h, next_qt, pT))
                next_qt += 1
    while pvq:
        pop_pv()


def _build():
    import concourse.mybir as mybir
    import concourse.tile as tile
    from concourse import bacc
    from contextlib import ExitStack

    nc = bacc.Bacc()
    q = nc.declare_dram_parameter("q", [SEQ, QH * D], mybir.dt.float32, isOutput=False)
    k = nc.declare_dram_parameter("k", [SEQ, D], mybir.dt.float32, isOutput=False)
    v = nc.declare_dram_parameter("v", [SEQ, D], mybir.dt.float32, isOutput=False)
    out = nc.declare_dram_parameter("out", [SEQ, QH * D], mybir.dt.float32, isOutput=True)

    with tile.TileContext(nc) as tc:
        with ExitStack() as ctx:
            _emit(ctx, tc, q[:], k[:], v[:], out[:])
    nc.compile()
    return nc


def _get_nc():
    global _NC
    if _NC is None:
        _NC = _build()
    return _NC


def _ensure_ntff_hook():
    """The agent image's antenv lacks axon_hooks; shim it so trace=True works."""
    import sys
    import types

    if "antenv.axon_hooks" in sys.modules:
        return
    try:
        import antenv
        from trn_agent_boot.trn_boot import _ntff_profile_via_ctypes
    except ImportError:
        return
    mod = types.ModuleType("antenv.axon_hooks")
    hook = [None]
    mod.set_axon_ntff_profile_hook = lambda h: hook.__setitem__(0, h)
    mod.get_axon_ntff_profile_hook = lambda: hook[0]
    sys.modules["antenv.axon_hooks"] = mod
    antenv.axon_hooks = mod
    mod.set_axon_ntff_profile_hook(_ntff_profile_via_ctypes("/opt/axon/libaxon_pjrt.so"))


def _run(q, k, v, trace=False):
    from concourse.bass_utils import run_bass_kernel_spmd

    if trace:
        _ensure_ntff_hook()
    nc = _get_nc()
    in_maps = []
    for i in range(N_CORES):
        in_maps.append(
            {
                "q": np.ascontiguousarray(q[:, i * QH * D:(i + 1) * QH * D]).astype(np.float32, copy=False),
                "k": np.ascontiguousarray(k[:, i * D:(i + 1) * D]).astype(np.float32, copy=False),
                "v": np.ascontiguousarray(v[:, i * D:(i + 1) * D]).astype(np.float32, copy=False),
            }
        )
    res = run_bass_kernel_spmd(nc, in_maps, core_ids=list(range(N_CORES)), trace=trace)
    full = np.concatenate([res.results[i]["out"] for i in range(N_CORES)], axis=1)
    return full.astype(np.float32, copy=False), res


def kernel(q, k, v):
    out, _ = _run(q, k, v, trace=False)
    return out
